# revision 17
# baseline (speedup 1.0000x reference)
"""Trainium2 Bass kernel for nn_BPDecoder: logits = 1 - exp(-exp(sum_i R_i*||Z_i||^2)).

Strategy (8-core SPMD, row-sharded, bf16 on-device):
  - Pad N=500000 rows to 8 * 63488; core k takes rows [k*63488, (k+1)*63488).
  - Host casts Z and R to bf16 (measured end-to-end rel err ~1.3e-4).
  - Row->partition assignment is (tile, partition, q): each partition owns 16
    CONSECUTIVE rows per tile, so the DMA source runs are 16*128*2B = 4KB
    contiguous (>=512B line-rate minimum for bf16).  R is permuted on host to
    (tile, q, partition) order so the stationary matmul operand loads as a
    plain [128, T*Q] column block.
  - Z slabs of 2 tiles (1MB) round-robin over the three dynamic DMA rings
    (sync HWDGE / scalar HWDGE / gpsimd SWDGE) so per-DMA completion gaps
    overlap with other rings' transfers.
  - Squares (bf16) alternate between DVE (tensor_mul, 2x mode, ~1.13us/tile)
    and ACT (~1.89us/tile) at ~5:3 to balance; 4 PE matmuls per tile with the
    per-tile R block [128, 16] stationary accumulate
    C[q', (q, d)] += sum_p R[p, q'] * Z[p, q, d]^2 into one PSUM [16,2048] f32.
  - Host extracts/sums the diagonal blocks q' == q of the 8 small outputs and
    applies the scalar 1 - exp(-exp(s)) in f64.
"""

import sys

sys.path.insert(0, "/opt/trn_rl_repo")


# The agent image lacks antenv.axon_hooks; recreate it so trace=True works
# (bass_utils imports it lazily for NTFF profiling under axon).
def _install_ntff_hook_shim():
    import types
    if "antenv.axon_hooks" in sys.modules:
        return
    mod = types.ModuleType("antenv.axon_hooks")
    state = {"hook": None}
    mod.set_axon_ntff_profile_hook = lambda h: state.__setitem__("hook", h)
    mod.get_axon_ntff_profile_hook = lambda: state["hook"]
    sys.modules["antenv.axon_hooks"] = mod
    try:
        sys.path.insert(0, "/root/.axon_site")
        from trn_agent_boot.trn_boot import _ntff_profile_via_ctypes
        state["hook"] = _ntff_profile_via_ctypes("/opt/axon/libaxon_pjrt.so")
    except Exception:
        pass


_install_ntff_hook_shim()

import numpy as np

import concourse.bass as bass
import concourse.bacc as bacc
import concourse.mybir as mybir
from concourse.tile import TileContext
from concourse.bass_utils import run_bass_kernel_spmd

P = 128          # SBUF partitions
D = 128          # row length (feature dim)
Q = 16           # rows per partition per tile (consecutive)
FREE = Q * D     # free elems per tile = 2048
T = 31           # tiles per core
NC_ROWS = T * P * Q   # 63488 rows per core
N_CORES = 8
N_FULL = 500000
MM_N = 512       # matmul moving-operand slice (PSUM: <=512 f32 out per matmul)
NSLICES = FREE // MM_N
QS = Q // NSLICES     # q-groups per matmul slice

SLAB = 2         # tiles per DMA slab

Z_DT = mybir.dt.bfloat16
R_DT = mybir.dt.bfloat16
S_DT = mybir.dt.bfloat16   # dtype of the squared tile (matmul rhs)

# square-engine pattern, period 9 (True -> DVE, False -> ACT); ACT is
# effectively faster than DVE (no DRAIN), so ACT gets ~5:4
DVE_PATTERN = (False, True, False, True, False, True, False, True, False)

_cache = {}


def _np_dt(dt):
    return mybir.dt.np(dt)


def _build():
    nc = bacc.Bacc(trn_type="TRN2")
    z = nc.declare_dram_parameter("z", [NC_ROWS, D], Z_DT, isOutput=False)
    # r is host-permuted into the ready-to-load [128, T*Q] stationary matrix
    r = nc.declare_dram_parameter("r", [P, T * Q], R_DT, isOutput=False)
    out = nc.declare_dram_parameter("out", [Q, FREE], mybir.dt.float32, isOutput=True)

    # rows are laid out (t, p, q): partition p owns rows [t*P*Q + p*Q, +Q)
    z_view = z.rearrange("(t p q) d -> p t (q d)", p=P, q=Q)  # [128, T, 2048]
    r_cols = r[:]                                             # [128, T*Q]

    slabs = [(s, min(s + SLAB, T)) for s in range(0, T, SLAB)]
    dma_engines = ["sync", "scalar", "gpsimd"]

    with TileContext(nc) as tc:
        with (
            tc.tile_pool(name="zpool", bufs=6) as zpool,
            tc.tile_pool(name="spool", bufs=6) as spool,
            tc.tile_pool(name="singles", bufs=1) as singles,
            tc.tile_pool(name="ppool", bufs=1, space="PSUM") as ppool,
        ):
            r_sb = singles.tile([P, T * Q], R_DT)
            nc.scalar.dma_start(out=r_sb[:], in_=r_cols)

            acc = ppool.tile([Q, FREE], mybir.dt.float32)

            for si, (t0, t1) in enumerate(slabs):
                nt = t1 - t0
                z_sb = zpool.tile([P, SLAB, FREE], Z_DT, tag="z")
                eng = getattr(nc, dma_engines[si % len(dma_engines)])
                eng.dma_start(out=z_sb[:, :nt, :], in_=z_view[:, t0:t1, :])
                s_sb = spool.tile([P, SLAB, FREE], S_DT, tag="s")
                for t in range(t0, t1):
                    ti = t - t0
                    if DVE_PATTERN[t % len(DVE_PATTERN)]:
                        nc.vector.tensor_mul(
                            s_sb[:, ti, :], z_sb[:, ti, :], z_sb[:, ti, :]
                        )
                    else:
                        nc.scalar.square(s_sb[:, ti, :], z_sb[:, ti, :])
                    for sl in range(NSLICES):
                        nc.tensor.matmul(
                            acc[:, sl * MM_N:(sl + 1) * MM_N],
                            r_sb[:, t * Q:(t + 1) * Q],
                            s_sb[:, ti, sl * MM_N:(sl + 1) * MM_N],
                            start=(t == 0),
                            stop=(t == T - 1),
                        )

            out_sb = singles.tile([Q, FREE], mybir.dt.float32)
            nc.vector.tensor_copy(out_sb[:], acc[:])
            nc.sync.dma_start(out=out[:], in_=out_sb[:])
    nc.compile()
    return nc


def _get_nc():
    if "nc" not in _cache:
        _cache["nc"] = _build()
    return _cache["nc"]


def _shard(Z, R):
    np_z = _np_dt(Z_DT)
    np_r = _np_dt(R_DT)
    ZP = np.zeros((N_CORES, NC_ROWS, D), dtype=np_z)
    ZP.reshape(-1, D)[:N_FULL] = Z.astype(np_z, copy=False)
    RP = np.zeros((N_CORES, NC_ROWS), dtype=np_r)
    RP.reshape(-1)[:N_FULL] = R.astype(np_r, copy=False)
    # device loads R as a plain [128, T*Q] matrix: R_mat[p, t*Q+q] = R[t,p,q]
    RPerm = RP.reshape(N_CORES, T, P, Q).transpose(0, 2, 1, 3)
    RPerm = np.ascontiguousarray(RPerm).reshape(N_CORES, P, T * Q)
    return [{"z": ZP[k], "r": RPerm[k]} for k in range(N_CORES)]


def _combine(results):
    idx = np.arange(Q)
    s = 0.0
    for res in results:
        C = np.asarray(res["out"], dtype=np.float64).reshape(Q, Q, D)
        s += C[idx, idx, :].sum()
    lam = np.exp(s)
    logits = 1.0 - np.exp(-lam)
    return np.float32(logits)


def _run(Z, R, trace=False, tmpdir=None):
    nc = _get_nc()
    in_maps = _shard(Z, R)
    return run_bass_kernel_spmd(nc, in_maps, core_ids=list(range(N_CORES)),
                                trace=trace, tmpdir=tmpdir)


def kernel(Z, R):
    assert Z.shape == (N_FULL, D) and R.shape == (N_FULL,)
    out = _run(np.asarray(Z), np.asarray(R), trace=False)
    return _combine(out.results)
